# revision 32
# baseline (speedup 1.0000x reference)
"""Causal multi-head self-attention on 8 TRN2 NeuronCores.

Problem: x (2, 2048, 1024) f32; w_q/w_k/w_v/w_o (1024, 1024) f32;
out = CausalMHA(x) (torch nn.Linear convention, 16 heads, d_k = 64).

Sharding (tensor-parallel over heads x data-parallel over batch):
core c -> batch bc = c//4, head group hg = c%4 (4 heads = 256 features).
Each core computes Q/K/V projections for its slice, causal attention for
its 4 heads, and a partial output projection against its w_o column
slice. The host sums the 4 partials per batch (the tensor-parallel
"all-reduce" of the w_o matmul, done host-side during unshard).

Device kernel (per core, one NEFF, all matmuls bf16 w/ f32 PSUM accum):
- scores computed K-major (ST layout [k, q]) so softmax'd P lands
  pre-transposed for the P^T V matmul
- softmax without max subtraction (inputs bounded, exp args within +-6);
  exp on ScalarE reads straight from PSUM
- causal masking: band tiles use narrowed matmuls/exp plus a triangular
  affine_select zeroing on GPSIMD
- the two heads of a pair are issued to PE row groups (0,0)/(64,0) and
  run concurrently in the systolic array (K=64 each)
- AV stationary is [64 ones-cols | 64 V-cols] (M=128): PSUM rows 0-63
  accumulate 64 broadcast copies of the softmax row-sums while rows
  64-127 accumulate Y.  The matmul costs N streaming cycles regardless
  of M, so the partition-broadcast of the denominators is free; the
  whole normalize is then reciprocal_approx_fast(av[0:64]) (custom DVE
  op on a base_partition-0 AP, as HW requires) + one tensor_mul.
- projections / attention / output-projection are statically software-
  pipelined: proj and out-proj groups are emitted as filler between
  attention k-tile groups so TensorE stays dense
- x is DMAd with one descriptor per 512-token chunk (first chunk in two
  halves), ordered wq, x0, wk, x1.. so the first proj starts early;
  V/O weights ride the separate GPSIMD SWDGE ring in parallel.
- output returned bf16 (cast to f32 host-side); error budget dominated
  by the bf16 input quantization (~4e-3 max-relative overall).
"""

import numpy as np
import ml_dtypes

import concourse.bass as bass
import concourse.tile as tile
from concourse import bacc, mybir
from concourse.bass import ts

P = 128
D = 1024          # d_model
T = 2048          # seq len
B = 2
NH = 4            # heads per core
DK = 64
F = NH * DK       # 256 local features
TC = 512          # token chunk (matmul N)
NCHUNK = T // TC  # 4
NTT = T // P      # 16 token tiles
KA = D // P       # 8 dmodel chunks
SCALE = 1.0 / np.sqrt(DK)

BF16 = mybir.dt.bfloat16
F32 = mybir.dt.float32
EXP = mybir.ActivationFunctionType.Exp


def build_nc():
    nc = bacc.Bacc(None, target_bir_lowering=False)
    with tile.TileContext(nc) as tc:
        with tc.tile_pool(name="dram", bufs=1, space="DRAM") as dram:
            xT = dram.tile((P, NCHUNK, KA, TC), BF16, kind="ExternalInput", name="xT", uniquify=False)
            wqT = dram.tile((P, KA, F), BF16, kind="ExternalInput", name="wqT", uniquify=False)
            wkT = dram.tile((P, KA, F), BF16, kind="ExternalInput", name="wkT", uniquify=False)
            wvT = dram.tile((P, KA, F), BF16, kind="ExternalInput", name="wvT", uniquify=False)
            woT = dram.tile((P, F // P, D), BF16, kind="ExternalInput", name="woT", uniquify=False)
            out = dram.tile((P, NTT, D), BF16, kind="ExternalOutput", name="out", uniquify=False)

            with tc.tile_pool(name="big", bufs=1) as big:
                xT_sb = big.tile([P, NCHUNK, KA, TC], BF16)
                wqT_sb = big.tile([P, KA, F], BF16)
                wkT_sb = big.tile([P, KA, F], BF16)
                wvT_sb = big.tile([P, KA, F], BF16)
                woT_sb = big.tile([P, F // P, D], BF16)
                QT_sb = big.tile([P, 2, T], BF16)   # head pair-major
                KT_sb = big.tile([P, 2, T], BF16)
                V_sb = big.tile([P, NTT, NH, 128], BF16)  # [ones(64) | V(64)]
                # per-chunk Y tiles: separate tiles so outproj bodies for
                # chunk c never pick up a false dependency on a later
                # chunk's normalize writes.
                YT_c = [big.tile([P, 2, TC], BF16, name=f"YT{c}") for c in range(NCHUNK)]

                # All loads on the sync ring, in consumption order, so the
                # DMA engine drains them with first-proj data first (x is
                # chunk-contiguous in DRAM -> 8KB descriptor lines).  The
                # GPSIMD ring stays free for the ones-memset.
                nc.sync.dma_start(out=wqT_sb[:, 0:4], in_=wqT[:, 0:4])
                nc.sync.dma_start(out=xT_sb[:, 0, 0:4], in_=xT[:, 0, 0:4])
                nc.sync.dma_start(out=wqT_sb[:, 4:8], in_=wqT[:, 4:8])
                nc.sync.dma_start(out=xT_sb[:, 0, 4:8], in_=xT[:, 0, 4:8])
                nc.sync.dma_start(out=wkT_sb[:], in_=wkT[:])
                nc.sync.dma_start(out=wvT_sb[:], in_=wvT[:])
                nc.sync.dma_start(out=xT_sb[:, 1], in_=xT[:, 1])
                nc.sync.dma_start(out=woT_sb[:], in_=woT[:])
                nc.sync.dma_start(out=xT_sb[:, 2], in_=xT[:, 2])
                nc.sync.dma_start(out=xT_sb[:, 3], in_=xT[:, 3])
                nc.gpsimd.memset(V_sb[:, :, :, 0:64], 1.0)

                with (tc.tile_pool(name="flex", bufs=2, space="PSUM") as flexp,
                      tc.tile_pool(name="st", bufs=2, space="PSUM") as stp,
                      tc.tile_pool(name="av", bufs=2, space="PSUM") as avp,
                      tc.tile_pool(name="pt", bufs=4) as ptp,
                      tc.tile_pool(name="sm", bufs=4) as smp,
                      tc.tile_pool(name="warm", bufs=1) as warmp,
                      tc.tile_pool(name="ob", bufs=2) as obp):

                    if True:  # pre-warm ACT exp table during DMA phase
                        wt = warmp.tile([1, 8], F32)
                        nc.vector.memset(wt[:], 0.0)
                        nc.scalar.activation(wt[:], wt[:], EXP, scale=1.0)

                    if True:
                        # warm-up matmuls on zeroed SBUF while the x DMAs
                        # land: ~3.4us of sustained PE activity flips the
                        # HAM clock gate to 8/8 before the first real
                        # matmul, which otherwise runs its first ~4us at
                        # 1.2 GHz.
                        wsrc = warmp.tile([P, TC], BF16)
                        nc.vector.memset(wsrc[:], 0.0)
                        wps = stp.tile([P, 2, TC], F32, name="st_ps")
                        for _ in range(8):
                            nc.tensor.matmul(
                                wps[:, 0, :], lhsT=wsrc[:, 0:P], rhs=wsrc[:],
                                start=True, stop=True)
                        # read the warm tile: a reader-less PSUM tile gives
                        # the pool no WAR edge, so the scheduler could
                        # interleave the first real score matmuls with the
                        # warm-up group on the same banks (observed as a
                        # nondeterministic correctness race).
                        nc.vector.tensor_copy(wt[:], wps[0:1, 0, 0:8])

                    def qk_group(n, pr, which):
                        w_sb = wqT_sb if which == "q" else wkT_sb
                        dst = QT_sb if which == "q" else KT_sb
                        def emit():
                            ps = flexp.tile([P, TC], F32, name="flex")
                            for a in range(KA):
                                nc.tensor.matmul(
                                    ps[:], lhsT=w_sb[:, a, ts(pr, P)],
                                    rhs=xT_sb[:, n, a, :],
                                    start=(a == 0), stop=(a == KA - 1))
                            nc.vector.tensor_copy(dst[:, pr, ts(n, TC)], ps[:])
                        return emit

                    def v_group(tt):
                        def emit():
                            ps_v = flexp.tile([P, TC], F32, name="flex")
                            for a in range(KA):
                                nc.tensor.matmul(
                                    ps_v[:, 0:F], lhsT=xT_sb[:, tt // 4, a, ts(tt % 4, P)],
                                    rhs=wvT_sb[:, a, :],
                                    start=(a == 0), stop=(a == KA - 1))
                            nc.vector.tensor_copy(
                                V_sb[:, tt, :, 64:128],
                                ps_v[:, 0:F].rearrange("p (h d) -> p h d", h=NH))
                        return emit

                    def proj_groups(n):
                        gs = [qk_group(n, pr, w) for pr in range(2) for w in ("q", "k")]
                        gs += [v_group(tt) for tt in range(4 * n, 4 * n + 4)]
                        return gs

                    def proj(n):
                        for g in proj_groups(n):
                            g()

                    filler = []
                    _ob_cache = [None]
                    _reserve = [0]

                    def emit_filler(k=1, force=False):
                        for _ in range(k):
                            if filler and (force or len(filler) > _reserve[0]):
                                filler.pop(0)()

                    def normalize(av_ps, r, pr, n, mul_eng=None):
                        # av rows 0:64 = 64 broadcast copies of the row
                        # sums (ones block of the AV stationary); rows
                        # 64:128 = unnormalized Y.  Custom DVE op reads a
                        # base_partition-0 AP as real HW requires.
                        rec = smp.tile([64, TC], F32, name="rec")
                        nc.vector.reciprocal_approx_fast(out=rec[:], in_=av_ps[0:64, :])
                        (mul_eng or nc.vector).tensor_mul(
                            YT_c[n][r:r + 64, pr, :],
                            av_ps[64:128, :],
                            rec[:])

                    def attention(n):
                        # head pairs processed together: the two K=64 score
                        # matmuls go to distinct PE row groups (0,0)/(64,0)
                        # and run concurrently in the array.
                        last_kt = 4 * n + 3
                        ngroups = 2 * (4 * n + 4)
                        if n == NCHUNK - 1:
                            # reserve two filler bodies: they flush right
                            # before the last normalize is emitted, keeping
                            # the PE busy (and HAM warm) across the
                            # normalize window.
                            _reserve[0] = 2
                            stride = max(1, ngroups // max(1, len(filler) - 2))
                        else:
                            _reserve[0] = 0
                            stride = max(1, ngroups // max(1, len(filler)))
                        gi = 0
                        for hp in range(2):
                            av_a = avp.tile([P, TC], F32, name="av_ps")
                            av_b = avp.tile([P, TC], F32, name="av_ps")
                            for kt in range(4 * n + 4):
                                jj = kt - 4 * n
                                band = (jj >= 0)
                                s = 128 * jj if band else 0
                                st_ps = stp.tile([P, 2, TC], F32, name="st_ps")
                                pt_sb = ptp.tile([P, 2, TC], BF16, name="pt_sb")
                                for j, r in ((0, 0), (1, 64)):
                                    nc.tensor.matmul(
                                        st_ps[:, j, s:TC],
                                        lhsT=KT_sb[r:r + 64, hp, ts(kt, P)],
                                        rhs=QT_sb[r:r + 64, hp, n * TC + s:(n + 1) * TC],
                                        start=True, stop=True)
                                # filler lands between the score matmuls and the
                                # exp-gated AV matmuls: TensorE chews it while
                                # ScalarE exponentiates, instead of head-of-line
                                # blocking on the AV wait.
                                gi += 1
                                if gi % stride == 0:
                                    emit_filler(1)
                                nc.scalar.activation(
                                    pt_sb[:, :, s:TC], st_ps[:, :, s:TC],
                                    EXP, scale=float(SCALE))
                                if band:
                                    # one call masks both heads: iota is
                                    # col - partition, independent of j.
                                    nc.gpsimd.affine_select(
                                        out=pt_sb[:, :, s:s + 128],
                                        in_=pt_sb[:, :, s:s + 128],
                                        compare_op=mybir.AluOpType.is_ge,
                                        fill=0.0, base=0,
                                        pattern=[[0, 2], [1, 128]],
                                        channel_multiplier=-1)
                                for j, av in ((0, av_a), (1, av_b)):
                                    nc.tensor.matmul(
                                        av[:, s:TC],
                                        lhsT=V_sb[:, kt, 2 * hp + j, :],
                                        rhs=pt_sb[:, j, s:TC],
                                        start=(kt == 0), stop=(kt == last_kt))
                            if hp == 1 and n == NCHUNK - 1:
                                # tail cover: flush the reserved filler and
                                # the first two outproj bodies' a=0 halves
                                # (they depend only on the pair-0 Y,
                                # normalized during this hp's attention) so
                                # the in-order PE queue has ~3us of real
                                # work across the final normalize chain.
                                emit_filler(len(filler), force=True)
                                for tt in (4 * n, 4 * n + 1):
                                    _tail_heads.append((tt, op_head(tt)))
                            normalize(av_a, 0, hp, n)
                            normalize(av_b, 64, hp, n)

                    def op_group(tt):
                        def emit():
                            _op_body(tt)
                        return emit

                    _tail_heads = []

                    def op_head(tt):
                        # a=0 accumulation halves on the idle score pool:
                        # they only read the pair-0 Y, so they can run
                        # during the final normalize.
                        yt = YT_c[tt // 4]
                        pair = stp.tile([P, 2, TC], F32, name="st_ps")
                        for half in range(2):
                            nc.tensor.matmul(
                                pair[:, half, :],
                                lhsT=yt[:, 0, ts(tt % 4, P)],
                                rhs=woT_sb[:, 0, ts(half, TC)],
                                start=True, stop=False)
                        return pair

                    def op_tail(tt, pair):
                        yt = YT_c[tt // 4]
                        for half in range(2):
                            nc.tensor.matmul(
                                pair[:, half, :],
                                lhsT=yt[:, 1, ts(tt % 4, P)],
                                rhs=woT_sb[:, 1, ts(half, TC)],
                                start=False, stop=True)
                        _op_out(tt, (pair[:, 0, :], pair[:, 1, :]), tail=True)

                    def outproj(n):
                        for tt, pair in _tail_heads:
                            op_tail(tt, pair)
                        for tt in range(4 * n + 2, 4 * n + 4):
                            op_tail(tt, op_head(tt))

                    def _op_body(tt, tail=False):
                        yt = YT_c[tt // 4]
                        pss = (flexp.tile([P, TC], F32, name="flex"),
                               flexp.tile([P, TC], F32, name="flex"))
                        for a in range(2):
                            for half, ps in ((0, pss[0]), (1, pss[1])):
                                nc.tensor.matmul(
                                    ps,
                                    lhsT=yt[:, a, ts(tt % 4, P)],
                                    rhs=woT_sb[:, a, ts(half, TC)],
                                    start=(a == 0), stop=(a == 1))
                        _op_out(tt, pss, tail=tail)

                    def _op_out(tt, pss, tail=False):
                        # output staged per PAIR of token tiles so each DMA
                        # moves 512KB with 4KB per-partition lines (one
                        # 128KB DMA per half-tile is descriptor- and
                        # line-inefficient and backs up the ring at the
                        # kernel tail).
                        if tt % 2 == 0:
                            o_sb = obp.tile([P, 2, 2, TC], BF16, name="o_sb")
                            _ob_cache[0] = o_sb
                        else:
                            o_sb = _ob_cache[0]
                        for half in range(2):
                            if tail and half == 1:
                                # split tail casts across DVE and the (now
                                # idle) ScalarE so the drain after the last
                                # matmul isn't DVE-serialized.
                                nc.scalar.copy(o_sb[:, tt % 2, half, :], pss[half])
                            else:
                                nc.vector.tensor_copy(o_sb[:, tt % 2, half, :], pss[half])
                        if tt % 2 == 1:
                            nc.sync.dma_start(
                                out=out[:, tt - 1:tt + 1, :],
                                in_=o_sb[:].rearrange("p i a t -> p i (a t)"))

                    proj(0)
                    for n in range(NCHUNK):
                        if n + 1 < NCHUNK:
                            filler.extend(proj_groups(n + 1))
                        attention(n)
                        if n < NCHUNK - 1:
                            filler.extend(op_group(tt) for tt in range(4 * n, 4 * n + 4))
                        else:
                            emit_filler(len(filler))
                            outproj(n)
                    emit_filler(len(filler))
    nc.compile()
    return nc




# ---------------- host-side shard / gather + entry point ----------------

_NC_CACHE = []


def _part(a, p=P):
    """(p*chunks, rest...) -> (p, chunks, rest...) with partition inner."""
    k, rest = a.shape[0], a.shape[1:]
    return np.ascontiguousarray(
        a.reshape(k // p, p, *rest).transpose(1, 0, *range(2, a.ndim + 1)))


def _shard_inputs(x, w_q, w_k, w_v, w_o):
    bf = ml_dtypes.bfloat16
    in_maps = []
    # xT: (P, NCHUNK, KA, TC) — chunk-major so each chunk is one DMA with
    # 8KB contiguous per-partition lines.
    xT_b = [
        np.ascontiguousarray(
            _part(np.ascontiguousarray(np.asarray(x)[b].T).astype(bf))
            .reshape(P, KA, NCHUNK, TC).transpose(0, 2, 1, 3))
        for b in range(B)
    ]
    w_q, w_k, w_v, w_o = (np.asarray(w) for w in (w_q, w_k, w_v, w_o))
    for c in range(8):
        bc, hg = c // 4, c % 4
        r0 = hg * F
        in_maps.append({
            "xT": xT_b[bc],
            "wqT": _part(np.ascontiguousarray(w_q[r0:r0 + F].T).astype(bf)),
            "wkT": _part(np.ascontiguousarray(w_k[r0:r0 + F].T).astype(bf)),
            "wvT": _part(np.ascontiguousarray(w_v[r0:r0 + F].T).astype(bf)),
            "woT": _part(np.ascontiguousarray(w_o[:, r0:r0 + F].T).astype(bf)),
        })
    return in_maps


def _gather(results):
    out = np.zeros((B, T, D), np.float32)
    for c in range(8):
        bc = c // 4
        part = np.asarray(results[c]["out"]).astype(np.float32).reshape(P, NTT, D)
        out[bc] += part.transpose(1, 0, 2).reshape(T, D)
    return out


def kernel(x, w_q, w_k, w_v, w_o):
    from concourse.bass_utils import run_bass_kernel_spmd
    if not _NC_CACHE:
        _NC_CACHE.append(build_nc())
    nc = _NC_CACHE[0]
    in_maps = _shard_inputs(x, w_q, w_k, w_v, w_o)
    res = run_bass_kernel_spmd(nc, in_maps, core_ids=list(range(8)))
    return _gather(res.results)
